# revision 21
# baseline (speedup 1.0000x reference)
"""Trainium2 Bass kernel for nn_Aggregator (GNN message passing).

Strategy (8 NeuronCores, SPMD — one program, per-core data):
  - Entities partitioned by id range: core c owns heads [c*12500, (c+1)*12500).
  - Users partitioned by id range: core c owns users [c*6250, (c+1)*6250).
  - Edges / interactions are sharded to the core owning their head / row and
    sorted by (8-window PSUM block, tail chunk, 128-wide window) so segment
    sums become one-hot matmuls accumulated in PSUM.
  - ent[tail] / ent[col] rows fetched with dma_gather (int16 indices against
    25000-row chunks of the entity table).
  - Phase A on device computes S = sigmoid(ent_core @ w.T); the host only
    replicates those values per edge (pure indexing) and feeds them to the
    main program, where they fold into the one-hot build
    (tensor_scalar is_equal then mult).
  - w[rel] rows per edge expanded by a K=16 one-hot matmul (host builds the
    0/1 relation one-hot from the integer edge_type input).
  - scatter-mean division folded into the PSUM flush (ACT copy with
    per-partition scale); user gate folded into the user flush.
  - Outputs are range-disjoint across cores: no collectives needed.

Host-side work is index preprocessing only (sharding, sorting, padding,
one-hot/count construction from the integer index tensors, and replication
of device-computed attention values per edge) — all floating-point
arithmetic on tensor data runs on device.
"""
import os
import sys
from contextlib import ExitStack

import numpy as np

for _p in ("/opt/trn_rl_repo", "/root/.axon_site/_ro/trn_rl_repo"):
    if os.path.isdir(_p) and _p not in sys.path:
        sys.path.insert(0, _p)

import concourse.bass as bass  # noqa: E402
import concourse.bacc as bacc  # noqa: E402
import concourse.tile as tile  # noqa: E402
from concourse import mybir  # noqa: E402
from concourse.bass_utils import run_bass_kernel_spmd  # noqa: E402
from concourse.masks import make_identity  # noqa: E402

dt = mybir.dt
AF = mybir.ActivationFunctionType
OP = mybir.AluOpType

P = 128
C = 64
R = 16
F = 4
N_ENTITIES = 100000
N_USERS = 50000
NCORES = 8
EPC = N_ENTITIES // NCORES      # 12500 heads per core
UPC = N_USERS // NCORES         # 6250 users per core
EW = (EPC + P - 1) // P         # 98 entity windows
UW = (UPC + P - 1) // P         # 49 user windows
CHUNK = 25000                   # entity-table rows per dma_gather chunk
NCHUNK = (N_ENTITIES + CHUNK - 1) // CHUNK
GRP = 8                         # tiles per We-batch
BLKW = 6                        # windows in flight (one PSUM bank each)
# Max indices per dma_gather call. 512 is HW-validated; >=2048 crashes the
# device (NRT_EXEC_UNIT_UNRECOVERABLE) — do not raise without re-validating.
GSUB = int(os.environ.get("KGSUB", "512"))

TRACE = os.environ.get("KBENCH_TRACE", "0") == "1"
DBG_SKIP_A = os.environ.get("KDBG_SKIP_A", "0") == "1"
DBG_EBLKS = int(os.environ.get("KDBG_EBLKS", "-1"))
DBG_UBLKS = int(os.environ.get("KDBG_UBLKS", "-1"))
DBG_NOGATHER = os.environ.get("KDBG_NOGATHER", "0") == "1"
LAST_RESULTS = {}


# --------------------------------------------------------------------------
# host-side prep (index preprocessing only)
# --------------------------------------------------------------------------
def _group_layout(w, k, n_win, tiles):
    """Slot index per element for the (block, chunk, window) sort order with
    per-(window, chunk) padding to tiles[w][k]*128 slots.

    Returns (order, slots, seg) where seg[(b, k)] = (tile_start, n_tiles).
    """
    b = w // BLKW
    order = np.lexsort((w, k, b))
    gid = w * NCHUNK + k
    # slot base per group in (block, chunk, window) order
    base = np.zeros(n_win * NCHUNK, np.int64)
    seg = {}
    pos = 0
    nblk = (n_win + BLKW - 1) // BLKW
    for bb in range(nblk):
        wlo, whi = bb * BLKW, min((bb + 1) * BLKW, n_win)
        for kk in range(NCHUNK):
            seg_start = pos
            for ww in range(wlo, whi):
                base[ww * NCHUNK + kk] = pos
                pos += tiles[ww][kk] * P
            seg[(bb, kk)] = (seg_start // P, (pos - seg_start) // P)
    gs_sorted = gid[order]
    # (block, chunk, window) sorting makes each group a contiguous run
    change = np.concatenate([[True], gs_sorted[1:] != gs_sorted[:-1]])
    run_id = np.cumsum(change) - 1
    run_starts = np.concatenate(
        [[0], np.cumsum(np.bincount(run_id))])[:-1]
    within = np.arange(len(gs_sorted)) - run_starts[run_id]
    slots = base[gs_sorted] + within
    return order, slots, seg, pos


def _tiles_matrix(parts, n_win):
    """tiles[w][k] = max over cores of ceil(count(w, k)/128); ensures every
    window has at least one tile somewhere."""
    tiles = np.zeros((n_win, NCHUNK), np.int64)
    for (wv, kv) in parts:
        cnt = np.bincount(wv * NCHUNK + kv,
                          minlength=n_win * NCHUNK).reshape(n_win, NCHUNK)
        tiles = np.maximum(tiles, (cnt + P - 1) // P)
    empty = tiles.sum(axis=1) == 0
    tiles[empty, 0] = 1
    return tiles


def _to_pt(slots, values, total, fill, dtype):
    a = np.full(total, fill, dtype=dtype)
    a[slots] = values
    return np.ascontiguousarray(a.reshape(total // P, P).T)  # [128, T]


def _subcalls(t0, ntl):
    """Split a segment of ntl tiles into gather sub-calls of <= GSUB idx."""
    step = GSUB // P
    out = []
    while ntl > 0:
        n = min(step, ntl)
        out.append((t0, n))
        t0 += n
        ntl -= n
    return out


def _wrap_idx(flat, seg_list):
    """Wrap chunk-relative indices into the dma_gather int16 layout:
    per gather sub-call, index i lands at [i % 16, i // 16], replicated to
    all 8 groups of 16 partitions. flat: [total] int16 in slot order.
    seg_list: [(tile_start, n_tiles)] segments, each split into sub-calls."""
    cols = []
    for (s0, sn) in seg_list:
        for (t0, ntl) in _subcalls(s0, sn):
            n = ntl * P
            blk = flat[t0 * P:t0 * P + n]
            wrapped = blk.reshape(n // 16, 16).T  # [16, n/16]
            cols.append(np.tile(wrapped, (8, 1)))
    return np.ascontiguousarray(np.concatenate(cols, axis=1))


def _prep(inputs):
    ent = np.ascontiguousarray(np.asarray(inputs["entity_emb"], np.float32))
    usr = np.asarray(inputs["user_emb"], np.float32)
    lat = np.asarray(inputs["latent_emb"], np.float32)
    wgt = np.ascontiguousarray(np.asarray(inputs["weight"], np.float32))
    dwa = np.asarray(inputs["disen_weight_att"], np.float32)
    ivals = np.asarray(inputs["interact_vals"], np.float32)
    head = np.asarray(inputs["head"], np.int64)
    tail = np.asarray(inputs["tail"], np.int64)
    etype = np.asarray(inputs["edge_type"], np.int64)
    irows = np.asarray(inputs["interact_rows"], np.int64)
    icols = np.asarray(inputs["interact_cols"], np.int64)

    e_core = head // EPC
    u_core = irows // UPC
    e_parts, u_parts = [], []
    for c in range(NCORES):
        m = e_core == c
        h = head[m] - c * EPC
        e_parts.append((h, tail[m], etype[m] - 1))
        m = u_core == c
        u_parts.append((irows[m] - c * UPC, icols[m], ivals[m]))

    etiles = _tiles_matrix(
        [((h // P), t // CHUNK) for (h, t, r) in e_parts], EW)
    utiles = _tiles_matrix(
        [((u // P), t // CHUNK) for (u, t, v) in u_parts], UW)

    per_core = []
    for c in range(NCORES):
        h, t, r = e_parts[c]
        order, slots, eseg, e_total = _group_layout(
            h // P, t // CHUNK, EW, etiles)
        h, t, r = h[order], t[order], r[order]
        lh_pt = _to_pt(slots, (h % P).astype(np.float32), e_total, 0.0,
                       np.float32)
        # per-edge (p, w, r) triple for host-side att expansion after phase A
        ap_slot = np.zeros(e_total, np.int64)
        ap_slot[slots] = (h % P) * (EW * R) + (h // P) * R + r
        ap_valid = np.zeros(e_total, bool)
        ap_valid[slots] = True
        tflat = np.zeros(e_total, np.int16)
        tflat[slots] = (t - (t // CHUNK) * CHUNK).astype(np.int16)
        eseg_list = [eseg[key] for key in sorted(eseg)]
        tidx_w = _wrap_idx(tflat, eseg_list)
        orT = np.zeros((R, e_total), np.float32)
        orT[r, slots] = 1.0
        cnt_h = np.bincount(h, minlength=EW * P).astype(np.float32)
        recip = 1.0 / np.maximum(cnt_h, 1.0)
        recip[EPC:] = 0.0
        recip_pw = np.ascontiguousarray(recip.reshape(EW, P).T)

        u, uc, v = u_parts[c]
        order, slots, useg, u_total = _group_layout(
            u // P, uc // CHUNK, UW, utiles)
        u, uc, v = u[order], uc[order], v[order]
        lu_pt = _to_pt(slots, (u % P).astype(np.float32), u_total, 0.0,
                       np.float32)
        val_pt = _to_pt(slots, v.astype(np.float32), u_total, 0.0, np.float32)
        cflat = np.zeros(u_total, np.int16)
        cflat[slots] = (uc - (uc // CHUNK) * CHUNK).astype(np.int16)
        useg_list = [useg[key] for key in sorted(useg)]
        cidx_w = _wrap_idx(cflat, useg_list)

        entT = np.zeros((C, EW * P), np.float32)
        entT[:, :EPC] = ent[c * EPC:(c + 1) * EPC].T
        u_pad = np.zeros((UW * P, C), np.float32)
        u_pad[:UPC] = usr[c * UPC:(c + 1) * UPC]
        u_pwc = np.ascontiguousarray(
            u_pad.reshape(UW, P, C).transpose(1, 0, 2).reshape(P, UW * C))

        chunks = {f"ent{kk}": np.ascontiguousarray(
            ent[kk * CHUNK:min((kk + 1) * CHUNK, N_ENTITIES)])
            for kk in range(NCHUNK)}
        per_core.append(dict(
            **chunks, entT=entT, w=wgt, wT=np.ascontiguousarray(wgt.T),
            dwa=dwa, latent=np.ascontiguousarray(lat.reshape(1, F * C)),
            usb=u_pwc, recip=recip_pw,
            lh=lh_pt, tidx=tidx_w, orT=orT,
            lu=lu_pt, cidx=cidx_w, val=val_pt,
            _aidx=ap_slot, _avalid=ap_valid,
        ))
    return per_core, etiles.tolist(), utiles.tolist()


# --------------------------------------------------------------------------
# phase A: compute S = sigmoid(ent_core @ w.T) on device
# --------------------------------------------------------------------------
def _build_phase_a():
    nc = bacc.Bacc("TRN2", target_bir_lowering=False, debug=False)
    f32 = dt.float32
    d_entT = nc.declare_dram_parameter("entT", [C, EW * P], f32,
                                       isOutput=False)
    d_wT = nc.declare_dram_parameter("wT", [C, R], f32, isOutput=False)
    d_s = nc.declare_dram_parameter("s_out", [P, EW * R], f32, isOutput=True)

    with tile.TileContext(nc) as tc, ExitStack() as ctx:
        sb = ctx.enter_context(tc.tile_pool(name="sb", bufs=2))
        psp = ctx.enter_context(tc.tile_pool(name="psp", bufs=2,
                                             space="PSUM"))
        wT_sb = sb.tile([C, R], f32, tag="wt", bufs=1)
        nc.sync.dma_start(out=wT_sb[:], in_=d_wT[:])
        s_sb = sb.tile([P, EW * R], f32, tag="s", bufs=1)
        for w0 in range(0, EW, GRP):
            nwin = min(GRP, EW - w0)
            entT_t = sb.tile([C, GRP * P], f32, tag="entT")
            nc.sync.dma_start(out=entT_t[:, :nwin * P],
                              in_=d_entT[:, w0 * P:(w0 + nwin) * P])
            sp = psp.tile([P, GRP * R], f32, space="PSUM", tag="sps")
            for k in range(nwin):
                nc.tensor.matmul(out=sp[:, k * R:(k + 1) * R],
                                 lhsT=entT_t[:, k * P:(k + 1) * P],
                                 rhs=wT_sb[:], start=True, stop=True)
            nc.scalar.activation(s_sb[:, w0 * R:(w0 + nwin) * R],
                                 sp[:, :nwin * R], AF.Sigmoid)
        nc.sync.dma_start(out=d_s[:], in_=s_sb[:])
    nc.compile()
    return nc


# --------------------------------------------------------------------------
# main program
# --------------------------------------------------------------------------
def _win_schedule(tiles):
    """Per-tile (window, first, last) + per-(block, chunk) segment sizes in
    emission order."""
    n_win = len(tiles)
    nblk = (n_win + BLKW - 1) // BLKW
    sched = []
    segs = []
    remaining = [sum(tiles[w]) for w in range(n_win)]
    seen = [0] * n_win
    for b in range(nblk):
        wlo, whi = b * BLKW, min((b + 1) * BLKW, n_win)
        for k in range(NCHUNK):
            n_seg = sum(tiles[w][k] for w in range(wlo, whi))
            segs.append((b, k, n_seg))
            for w in range(wlo, whi):
                for _ in range(tiles[w][k]):
                    seen[w] += 1
                    sched.append((w, seen[w] == 1, seen[w] == remaining[w]))
    return sched, segs


def _build_main(etiles, utiles):
    e_total = sum(sum(tw) for tw in etiles) * P
    u_total = sum(sum(tw) for tw in utiles) * P
    TE = e_total // P
    TU = u_total // P

    nc = bacc.Bacc("TRN2", target_bir_lowering=False, debug=False)
    f32, i16 = dt.float32, dt.int16
    D = {}
    chunk_specs = [(f"ent{k}", [min((k + 1) * CHUNK, N_ENTITIES) - k * CHUNK,
                                C], f32) for k in range(NCHUNK)]
    for name, shape, dtp in chunk_specs + [
        ("w", [R, C], f32),
        ("dwa", [F, R], f32),
        ("latent", [1, F * C], f32),
        ("usb", [P, UW * C], f32),
        ("recip", [P, EW], f32),
        ("lh", [P, TE], f32),
        ("att", [P, TE], f32),
        ("tidx", [P, TE * P // 16], i16),
        ("orT", [R, e_total], f32),
        ("lu", [P, TU], f32),
        ("val", [P, TU], f32),
        ("cidx", [P, TU * P // 16], i16),
    ]:
        D[name] = nc.declare_dram_parameter(name, shape, dtp, isOutput=False)
    out_ent = nc.declare_dram_parameter("out_ent", [P, EW * C], f32,
                                        isOutput=True)
    out_usr = nc.declare_dram_parameter("out_usr", [P, UW * C], f32,
                                        isOutput=True)
    dis_scr = nc.dram_tensor("dis_scr", [F, C], f32)

    esched, esegs = _win_schedule(etiles)
    usched, usegs = _win_schedule(utiles)
    emax_seg = max(n for (_, _, n) in esegs)
    umax_seg = max(n for (_, _, n) in usegs)
    max_seg = max(emax_seg, umax_seg)

    with tile.TileContext(nc) as tc, ExitStack() as ctx:
        cst = ctx.enter_context(tc.tile_pool(name="cst", bufs=1))
        sb = ctx.enter_context(tc.tile_pool(name="sb", bufs=1))
        ps = ctx.enter_context(tc.tile_pool(name="ps", bufs=2, space="PSUM"))
        pswe = ctx.enter_context(tc.tile_pool(name="pswe", bufs=2,
                                              space="PSUM"))
        st3 = ctx.enter_context(tc.tile_pool(name="st3", bufs=3))
        st2 = ctx.enter_context(tc.tile_pool(name="st2", bufs=2))

        # ---------------- constants / small tables ----------------
        iota_i = cst.tile([P, P], dt.int32)
        iota_f = cst.tile([P, P], f32)
        nc.gpsimd.iota(iota_i[:], pattern=[[1, P]], base=0,
                       channel_multiplier=0)
        nc.vector.tensor_copy(iota_f[:], iota_i[:])
        ident = cst.tile([P, P], f32)
        make_identity(nc, ident[:])
        ones_row = cst.tile([1, P], f32)
        nc.vector.memset(ones_row[:], 1.0)

        w_sb = cst.tile([R, C], f32)
        nc.sync.dma_start(out=w_sb[:], in_=D["w"][:])
        dwa_sb = cst.tile([F, R], f32)
        nc.sync.dma_start(out=dwa_sb[:], in_=D["dwa"][:])
        lat_row = cst.tile([1, F * C], f32)
        nc.sync.dma_start(out=lat_row[:], in_=D["latent"][:])
        recip_sb = cst.tile([P, EW], f32)
        nc.sync.dma_start(out=recip_sb[:], in_=D["recip"][:])
        usb = cst.tile([P, UW, C], f32)
        nc.sync.dma_start(out=usb[:], in_=D["usb"][:])

        lat_ps = pswe.tile([P, F * C], f32, space="PSUM", tag="wep")
        nc.tensor.matmul(out=lat_ps[:], lhsT=ones_row[:], rhs=lat_row[:],
                         start=True, stop=True)
        lat_bc = cst.tile([P, F, C], f32)
        nc.vector.tensor_copy(lat_bc[:], lat_ps[:])

        # ---------------- disen = softmax(dwa) @ w ----------------
        mx = sb.tile([F, 1], f32)
        nc.vector.tensor_reduce(out=mx[:], in_=dwa_sb[:],
                                axis=mybir.AxisListType.X, op=OP.max)
        nmx = sb.tile([F, 1], f32)
        nc.vector.tensor_scalar(out=nmx[:], in0=mx[:], scalar1=-1.0,
                                scalar2=None, op0=OP.mult)
        ex = sb.tile([F, R], f32)
        nc.scalar.activation(ex[:], dwa_sb[:], AF.Exp, bias=nmx[:], scale=1.0)
        sm = sb.tile([F, 1], f32)
        nc.vector.tensor_reduce(out=sm[:], in_=ex[:],
                                axis=mybir.AxisListType.X, op=OP.add)
        rsm = sb.tile([F, 1], f32)
        nc.vector.reciprocal(rsm[:], sm[:])
        smx = sb.tile([F, R], f32)
        nc.vector.tensor_scalar(out=smx[:], in0=ex[:], scalar1=rsm[:],
                                scalar2=None, op0=OP.mult)
        smxT_ps = pswe.tile([R, F], f32, space="PSUM", tag="wep")
        nc.tensor.transpose(out=smxT_ps[:], in_=smx[:], identity=ident[:F, :F])
        smxT = sb.tile([R, F], f32)
        nc.vector.tensor_copy(smxT[:], smxT_ps[:])
        dis_ps = pswe.tile([F, C], f32, space="PSUM", tag="wep")
        nc.tensor.matmul(out=dis_ps[:], lhsT=smxT[:], rhs=w_sb[:],
                         start=True, stop=True)
        dis_sb = sb.tile([F, C], f32)
        nc.vector.tensor_copy(dis_sb[:], dis_ps[:])
        nc.sync.dma_start(out=dis_scr[:], in_=dis_sb[:])
        dis_row = sb.tile([1, F * C], f32)
        nc.sync.dma_start(out=dis_row[:],
                          in_=dis_scr[:].rearrange("f (o c) -> o (f c)", o=1))
        dis_ps2 = pswe.tile([P, F * C], f32, space="PSUM", tag="wep")
        nc.tensor.matmul(out=dis_ps2[:], lhsT=ones_row[:], rhs=dis_row[:],
                         start=True, stop=True)
        dis_bc = cst.tile([P, F, C], f32)
        nc.vector.tensor_copy(dis_bc[:], dis_ps2[:])

        # ---------------- gate1 = 1 + softmax(U@latent.T) @ disen -----------
        score = sb.tile([P, F, UW], f32)
        usc = sb.tile([P, UW, C], f32, tag="uscratch")
        for f in range(F):
            nc.vector.tensor_tensor(
                out=usc[:], in0=usb[:],
                in1=lat_bc[:, f:f + 1, :].to_broadcast([P, UW, C]),
                op=OP.mult)
            nc.vector.tensor_reduce(out=score[:, f, :], in_=usc[:],
                                    axis=mybir.AxisListType.X, op=OP.add)
        esc = sb.tile([P, F, UW], f32)
        nc.scalar.activation(esc[:], score[:], AF.Exp)
        ssum = sb.tile([P, UW], f32)
        nc.vector.tensor_tensor(out=ssum[:], in0=esc[:, 0, :],
                                in1=esc[:, 1, :], op=OP.add)
        nc.vector.tensor_tensor(out=ssum[:], in0=ssum[:], in1=esc[:, 2, :],
                                op=OP.add)
        nc.vector.tensor_tensor(out=ssum[:], in0=ssum[:], in1=esc[:, 3, :],
                                op=OP.add)
        rs = sb.tile([P, UW], f32)
        nc.vector.reciprocal(rs[:], ssum[:])
        gate1 = cst.tile([P, UW, C], f32)
        for f in range(F):
            dst = gate1 if f == 0 else usc
            nc.vector.tensor_tensor(
                out=dst[:],
                in0=esc[:, f, :].rearrange("p (u o) -> p u o", o=1)
                    .to_broadcast([P, UW, C]),
                in1=dis_bc[:, f:f + 1, :].to_broadcast([P, UW, C]),
                op=OP.mult)
            if f > 0:
                nc.vector.tensor_tensor(out=gate1[:], in0=gate1[:],
                                        in1=usc[:], op=OP.add)
        nc.vector.tensor_tensor(
            out=gate1[:], in0=gate1[:],
            in1=rs[:].rearrange("p (u o) -> p u o", o=1).to_broadcast(
                [P, UW, C]),
            op=OP.mult)
        nc.vector.tensor_scalar(out=gate1[:], in0=gate1[:], scalar1=1.0,
                                scalar2=None, op0=OP.add)

        # ---------------- main segment loops ----------------
        out_ent_sb = cst.tile([P, EW, C], f32)
        out_usr_sb = cst.tile([P, UW, C], f32)

        def run_branch(tiles, sched, segs, ent_branch):
            n_win = len(tiles)
            pwin = {}
            t = 0
            seg_t0 = 0
            dbg_blks = DBG_EBLKS if ent_branch else DBG_UBLKS
            for (b, k, n_seg) in segs:
                if n_seg == 0:
                    continue
                if dbg_blks >= 0 and b >= dbg_blks:
                    t += n_seg
                    seg_t0 += n_seg
                    continue
                # gather this (block, chunk) segment
                idx_name = "tidx" if ent_branch else "cidx"
                ii = st2.tile([P, max_seg * P // 16], i16, tag="ii")
                nc.sync.dma_start(
                    out=ii[:, :n_seg * P // 16],
                    in_=D[idx_name][:, seg_t0 * P // 16:
                                    (seg_t0 + n_seg) * P // 16])
                g_t = st2.tile([P, max_seg, C], f32, tag="g")
                if DBG_NOGATHER:
                    nc.gpsimd.memset(g_t[:, :n_seg, :], 1.0)
                else:
                    for (c0, cn) in _subcalls(0, n_seg):
                        nc.gpsimd.dma_gather(
                            out_ap=g_t[:, c0:c0 + cn, :],
                            in_ap=D[f"ent{k}"][:],
                            idxs_ap=ii[:, c0 * P // 16:(c0 + cn) * P // 16],
                            num_idxs=cn * P, num_idxs_reg=cn * P,
                            elem_size=C)
                if ent_branch:
                    lh_t = st3.tile([P, max_seg], f32, tag="lh")
                    nc.sync.dma_start(out=lh_t[:, :n_seg],
                                      in_=D["lh"][:, seg_t0:seg_t0 + n_seg])
                    at_t = st3.tile([P, max_seg], f32, tag="at")
                    nc.sync.dma_start(out=at_t[:, :n_seg],
                                      in_=D["att"][:, seg_t0:seg_t0 + n_seg])
                    or_t = st2.tile([R, max_seg * P], f32, tag="or")
                    nc.sync.dma_start(
                        out=or_t[:, :n_seg * P],
                        in_=D["orT"][:, seg_t0 * P:(seg_t0 + n_seg) * P])
                else:
                    lh_t = st3.tile([P, max_seg], f32, tag="lh")
                    nc.sync.dma_start(out=lh_t[:, :n_seg],
                                      in_=D["lu"][:, seg_t0:seg_t0 + n_seg])
                    at_t = st3.tile([P, max_seg], f32, tag="at")
                    nc.sync.dma_start(out=at_t[:, :n_seg],
                                      in_=D["val"][:, seg_t0:seg_t0 + n_seg])
                    or_t = None
                # process tiles of this segment in groups of GRP
                for g0 in range(0, n_seg, GRP):
                    ng = min(GRP, n_seg - g0)
                    infos = []
                    for j in range(g0, g0 + ng):
                        w, first, last = sched[t + j]
                        if w not in pwin:
                            pwin[w] = ps.tile(
                                [P, C], f32, space="PSUM",
                                tag="segw", name=f"sg{ent_branch}_{w}",
                                bufs=BLKW)
                        infos.append((j, w, first, last, pwin[w]))
                    if ent_branch:
                        wep = pswe.tile([P, GRP * C], f32, space="PSUM",
                                        tag="wep")
                        vbuf = st3.tile([P, GRP * C], f32, tag="vbuf")
                        for kk, (j, w, first, last, pb) in enumerate(infos):
                            nc.tensor.matmul(
                                out=wep[:, kk * C:(kk + 1) * C],
                                lhsT=or_t[:, j * P:(j + 1) * P],
                                rhs=w_sb[:], start=True, stop=True)
                        nc.vector.tensor_tensor(
                            out=vbuf[:, :ng * C],
                            in0=g_t[:, g0:g0 + ng, :].rearrange(
                                "p t c -> p (t c)"),
                            in1=wep[:, :ng * C], op=OP.mult)
                    for kk, (j, w, first, last, pb) in enumerate(infos):
                        oh = st3.tile([P, P], f32, tag="oh")
                        nc.vector.tensor_scalar(
                            out=oh[:], in0=iota_f[:],
                            scalar1=lh_t[:, j:j + 1],
                            scalar2=at_t[:, j:j + 1],
                            op0=OP.is_equal, op1=OP.mult)
                        rhs = (vbuf[:, kk * C:(kk + 1) * C] if ent_branch
                               else g_t[:, j, :])
                        nc.tensor.matmul(
                            out=pb[:], lhsT=oh[:], rhs=rhs,
                            start=first, stop=last)
                        if last:
                            if ent_branch:
                                nc.scalar.activation(
                                    out_ent_sb[:, w, :], pb[:], AF.Copy,
                                    scale=recip_sb[:, w:w + 1])
                            else:
                                nc.vector.tensor_tensor(
                                    out=out_usr_sb[:, w, :], in0=pb[:],
                                    in1=gate1[:, w, :], op=OP.mult)
                            del pwin[w]
                t += n_seg
                seg_t0 += n_seg

        run_branch(etiles, esched, esegs, True)
        run_branch(utiles, usched, usegs, False)

        nc.sync.dma_start(out=out_ent[:],
                          in_=out_ent_sb[:].rearrange("p w c -> p (w c)"))
        nc.sync.dma_start(out=out_usr[:],
                          in_=out_usr_sb[:].rearrange("p w c -> p (w c)"))

    nc.compile()
    return nc


# --------------------------------------------------------------------------
def kernel(**inputs):
    per_core, etiles, utiles = _prep(inputs)

    if DBG_SKIP_A:
        class _Fake:
            results = [{"s_out": np.zeros((P, EW * R), np.float32)}
                       for _ in range(NCORES)]
        res_a = _Fake()
    else:
        nc_a = _build_phase_a()
        a_maps = [dict(entT=m["entT"], wT=m["wT"]) for m in per_core]
        res_a = run_bass_kernel_spmd(nc_a, a_maps, list(range(NCORES)))

    # host-side replication of device-computed att values per edge slot
    in_maps = []
    for c, m in enumerate(per_core):
        s_flat = res_a.results[c]["s_out"].reshape(-1)  # [p*EW*R + w*R + r]
        att = np.where(m["_avalid"], s_flat[m["_aidx"]], 0.0).astype(
            np.float32)
        total = att.shape[0]
        att_pt = np.ascontiguousarray(att.reshape(total // P, P).T)
        im = {k: v for k, v in m.items()
              if k not in ("_aidx", "_avalid", "entT", "wT")}
        im["att"] = att_pt
        in_maps.append(im)

    nc = _build_main(etiles, utiles)
    res = run_bass_kernel_spmd(nc, in_maps, list(range(NCORES)), trace=TRACE)
    LAST_RESULTS["res"] = res
    repeats = int(os.environ.get("KBENCH_REPEAT", "0"))
    if repeats:
        import time as _time
        walls = []
        for _ in range(repeats):
            t0 = _time.perf_counter()
            res = run_bass_kernel_spmd(nc, in_maps, list(range(NCORES)),
                                       trace=False)
            walls.append(_time.perf_counter() - t0)
        LAST_RESULTS["repeat_wall_s"] = min(walls)
        LAST_RESULTS["res"] = res

    ent_parts, usr_parts = [], []
    for c in range(NCORES):
        oe = res.results[c]["out_ent"].reshape(P, EW, C)
        ent_parts.append(oe.transpose(1, 0, 2).reshape(EW * P, C)[:EPC])
        ou = res.results[c]["out_usr"].reshape(P, UW, C)
        usr_parts.append(ou.transpose(1, 0, 2).reshape(UW * P, C)[:UPC])
    entity_agg = np.concatenate(ent_parts, axis=0)
    user_agg = np.concatenate(usr_parts, axis=0)
    return entity_agg, user_agg
